# revision 23
# baseline (speedup 1.0000x reference)
"""EntitiesAsExperts retrieval kernel for 8 Trainium2 NeuronCores.

Algorithm (entity table sharded along N across 8 cores):
  - host: gather mention spans from X, transpose small weights
  - device (SPMD, per core):
      pseudo = span @ Wf^T + b          (replicated small matmul)
      stream E shard in [128, C] chunks: scores chunk = pseudoT^T @ E_chunk (PE, fp32)
        per chunk: top-8 values (DVE max8) + their in-chunk positions (max_index)
      local top-104 of the per-chunk top-8s (13 rounds max8/match_replace)
      AllGather local top-100 values -> [8*256, 100]
      per mention: global max, 100th-largest (tau), softmax denominator over
        winners (v >= tau); alpha for own top-32 local candidates
      gather own winning entity rows (bf16 copy of shard, indirect DMA),
        weighted-accumulate -> partial picked; AllReduce picked
      upd = picked @ Wb^T + b; scatter rows into zeroed y at baked offsets
  - host: take core 0's y
"""

import os
import numpy as np
import ml_dtypes

import concourse.bass as bass
import concourse.bacc as bacc
import concourse.mybir as mybir
import concourse.tile as tile
from concourse.bass_utils import run_bass_kernel_spmd
from concourse.masks import make_identity

F32 = mybir.dt.float32
BF16 = mybir.dt.bfloat16
U16 = mybir.dt.uint16
U32 = mybir.dt.uint32
AX = mybir.AxisListType
ALU = mybir.AluOpType
ACTF = mybir.ActivationFunctionType

NEG = -1.0e30


class Cfg:
    def __init__(self, NCORE=8, B=8, S=512, DEMB=768, D=256, M=256, NL=62500,
                 C=500, K=100, CAP=28):
        self.NCORE, self.B, self.S, self.DEMB, self.D, self.M = NCORE, B, S, DEMB, D, M
        self.NL, self.C, self.K, self.CAP = NL, C, K, CAP
        self.AGK = CAP  # AllGather ships the rescored top-CAP; winners/core <= 26
        assert NL % C == 0
        self.G = NL // C
        self.G8 = self.G * 8
        self.MT = M // 128
        self.DT = D // 128
        self.ROUNDS = (max(self.AGK, CAP) + 7) // 8  # 4 -> 32 slots
        assert self.ROUNDS * 8 <= self.G8
        self.SPAN = 2 * DEMB  # 1536
        self.KT1 = self.SPAN // 128  # 12
        assert DEMB % 2 == 0
        self.NB = 2  # Wb output split
        assert CAP % 2 == 0 and CAP <= K


def build(cfg: Cfg, scatter_items, debug=False):
    """scatter_items: list of (row, mention) with unique rows (deduped, last-wins)."""
    c = cfg
    nc = bacc.Bacc(num_devices=c.NCORE)

    spanT = nc.declare_dram_parameter("spanT", [c.SPAN, c.M], F32, isOutput=False)
    WfT = nc.declare_dram_parameter("WfT", [c.SPAN, c.D], F32, isOutput=False)
    Wfb = nc.declare_dram_parameter("Wfb", [1, c.D], F32, isOutput=False)
    WbT = nc.declare_dram_parameter("WbT", [c.D, c.DEMB], F32, isOutput=False)
    Wbb = nc.declare_dram_parameter("Wbb", [1, c.DEMB], F32, isOutput=False)
    E = nc.declare_dram_parameter("E", [c.D, c.NL], F32, isOutput=False)
    ET = nc.declare_dram_parameter("ET", [c.NL, c.D], F32, isOutput=False)
    IOTA = nc.declare_dram_parameter("IOTA", [128, c.G8], F32, isOutput=False)
    ROWIDX = nc.declare_dram_parameter("ROWIDX", [c.M, 1], U32, isOutput=False)
    y = nc.declare_dram_parameter("y", [c.B * c.S, c.DEMB], F32, isOutput=True)
    if debug:
        dbg_lv = nc.declare_dram_parameter("dbg_lv", [c.M, c.ROUNDS * 8], F32, isOutput=True)
        dbg_lidx = nc.declare_dram_parameter("dbg_lidx", [c.M, c.CAP], U32, isOutput=True)
        dbg_alpha = nc.declare_dram_parameter("dbg_alpha", [c.M, c.CAP], F32, isOutput=True)
        dbg_acc = nc.declare_dram_parameter("dbg_acc", [c.M, c.D], F32, isOutput=True)
        dbg_pick = nc.declare_dram_parameter("dbg_pick", [c.M, c.D], F32, isOutput=True)
        dbg_pos = nc.declare_dram_parameter("dbg_pos", [c.M, c.ROUNDS * 8], U16, isOutput=True)
        dbg_i8 = nc.declare_dram_parameter("dbg_i8", [c.M, 8 * (62500 // 500 if c.NL == 62500 else c.NL // c.C)], U32, isOutput=True)

    groups = [list(range(c.NCORE))]

    with tile.TileContext(nc) as tc:
        with (
            tc.tile_pool(name="persist", bufs=1) as pp,
            tc.tile_pool(name="estream", bufs=3) as pe,
            tc.tile_pool(name="scratch", bufs=2) as psc,
            tc.tile_pool(name="psum", bufs=2, space="PSUM") as pps,
            tc.tile_pool(name="psum1", bufs=1, space="PSUM") as pps1,
            tc.tile_pool(name="dram", bufs=1, space="DRAM") as pd,
            tc.tile_pool(name="gather", bufs=2) as pg,
        ):
            ident = pp.tile([128, 128], F32)
            make_identity(nc, ident[:])
            ones1 = pp.tile([1, 128], F32)
            nc.vector.memset(ones1[:], 1.0)

            # ---- Stage A: pseudo = span @ Wf^T + b; pseudoT ----
            spanT_sb = [pp.tile([128, c.M], F32, tag=f"spanT{t}", name=f"spanT{t}") for t in range(c.KT1)]
            WfT_sb = [pp.tile([128, c.D], F32, tag=f"WfTs{t}", name=f"WfTs{t}") for t in range(c.KT1)]
            for t in range(c.KT1):
                spr = psc.tile([128, c.M], F32, tag="spr", name="spr")
                nc.sync.dma_start(out=spr[:], in_=spanT[t * 128:(t + 1) * 128, :])
                nc.vector.tensor_copy(out=spanT_sb[t][:], in_=spr[:])
                nc.sync.dma_start(out=WfT_sb[t][:], in_=WfT[t * 128:(t + 1) * 128, :])
            Wfb_sb = pp.tile([1, c.D], F32)
            nc.sync.dma_start(out=Wfb_sb[:], in_=Wfb[:])

            pseudo_sb = [pp.tile([128, c.D], F32, tag=f"pseudo{mt}", name=f"pseudo{mt}") for mt in range(c.MT)]
            for mt in range(c.MT):
                ps = pps1.tile([128, c.D], F32, tag="pseudo_ps", name="pseudo_ps")
                for t in range(c.KT1):
                    nc.tensor.matmul(
                        ps[:],
                        lhsT=spanT_sb[t][:, mt * 128:(mt + 1) * 128],
                        rhs=WfT_sb[t][:],
                        start=(t == 0),
                        stop=False,
                    )
                nc.tensor.matmul(
                    ps[:], lhsT=ones1[0:1, :], rhs=Wfb_sb[0:1, :],
                    start=False, stop=True,
                )
                nc.vector.tensor_copy(out=pseudo_sb[mt][:], in_=ps[:])

            pseudoT_sb = [pp.tile([128, c.M], mybir.dt.float32r, tag=f"pseudoT{dt}", name=f"pseudoT{dt}") for dt in range(c.DT)]
            for dt in range(c.DT):
                for mt in range(c.MT):
                    pst = pps1.tile([128, 128], F32, tag="tr_ps", name="tr_ps")
                    nc.tensor.transpose(
                        out=pst[:],
                        in_=pseudo_sb[mt][:, dt * 128:(dt + 1) * 128],
                        identity=ident[:],
                    )
                    nc.vector.tensor_copy(
                        out=pseudoT_sb[dt][:, mt * 128:(mt + 1) * 128], in_=pst[:]
                    )

            # ---- Stage B: stream E chunks; per-chunk top-8 + positions ----
            V8 = [pp.tile([128, c.G8], F32, tag=f"V8_{mt}", name=f"V8_{mt}") for mt in range(c.MT)]
            I8 = [pp.tile([128, c.G8], U32, tag=f"I8_{mt}", name=f"I8_{mt}") for mt in range(c.MT)]
            for ci in range(c.G):
                e01 = pe.tile([128, 2 * c.C], F32, tag="e01", name="e01")
                nc.sync.dma_start(out=e01[:, :c.C], in_=E[0:128, ci * c.C:(ci + 1) * c.C])
                nc.sync.dma_start(out=e01[:, c.C:], in_=E[128:256, ci * c.C:(ci + 1) * c.C])
                e01r = pe.tile([128, 2 * c.C], mybir.dt.float32r, tag="e01r", name="e01r")
                nc.scalar.copy(out=e01r[:], in_=e01[:])
                for mt in range(c.MT):
                    ps = pps.tile([128, c.C], F32, tag=f"score{mt}", name=f"score{mt}")
                    nc.tensor.matmul(
                        ps[:], lhsT=pseudoT_sb[0][:, mt * 128:(mt + 1) * 128],
                        rhs=e01r[:, :c.C], start=True, stop=False,
                    )
                    nc.tensor.matmul(
                        ps[:], lhsT=pseudoT_sb[1][:, mt * 128:(mt + 1) * 128],
                        rhs=e01r[:, c.C:], start=False, stop=True,
                    )
                    v8s = V8[mt][:, ci * 8:(ci + 1) * 8]
                    nc.vector.max(out=v8s, in_=ps[:])
                    nc.vector.max_index(
                        out=I8[mt][:, ci * 8:(ci + 1) * 8], in_max=v8s, in_values=ps[:]
                    )

            # ---- Stage C: local top-104 of V8; positions; top-CAP indices ----
            iotaf = pp.tile([128, c.G8], F32)
            nc.sync.dma_start(out=iotaf[:], in_=IOTA[:])
            W = [pp.tile([128, c.ROUNDS * 8], F32, tag=f"W{mt}", name=f"W{mt}") for mt in range(c.MT)]
            Pos = [pp.tile([128, c.ROUNDS * 8], U16, tag=f"Pos{mt}", name=f"Pos{mt}") for mt in range(c.MT)]
            lidx = [pp.tile([128, c.CAP], U32, tag=f"lidx{mt}", name=f"lidx{mt}") for mt in range(c.MT)]
            for mt in range(c.MT):
                work = pp.tile([128, c.G8], F32, tag=f"work{mt}", name=f"work{mt}")
                for r in range(c.ROUNDS):
                    src = V8[mt][:] if r == 0 else work[:]
                    w8 = W[mt][:, r * 8:(r + 1) * 8]
                    nc.vector.max(out=w8, in_=src)
                    nc.vector.max_index(out=Pos[mt][:, r * 8:(r + 1) * 8], in_max=w8, in_values=src)
                    if r < c.ROUNDS - 1:
                        nc.vector.match_replace(
                            out=work[:], in_to_replace=w8, in_values=src, imm_value=NEG
                        )
                # gather I8 at the top-CAP slots: per j, one-hot match on slot
                # index then reduce (indirect_copy shares idxs per 16-partition
                # group, so it cannot do per-mention gathers)
                i8f = pp.tile([128, c.G8], F32, tag=f"i8f{mt}", name=f"i8f{mt}")
                nc.vector.tensor_copy(out=i8f[:], in_=I8[mt][:])
                posf = pp.tile([128, c.CAP], F32, tag=f"posf{mt}", name=f"posf{mt}")
                nc.vector.tensor_copy(out=posf[:], in_=Pos[mt][:, :c.CAP])
                lidxf = pp.tile([128, c.CAP], F32, tag=f"lidxf{mt}", name=f"lidxf{mt}")
                for j in range(c.CAP):
                    mk = psc.tile([128, c.G8], F32, tag="mk", name="mk")
                    nc.vector.tensor_tensor(
                        out=mk[:], in0=iotaf[:],
                        in1=posf[:, j:j + 1].to_broadcast([128, c.G8]),
                        op=ALU.is_equal,
                    )
                    junk = psc.tile([128, c.G8], F32, tag="junk", name="junk")
                    nc.vector.tensor_tensor(
                        out=junk[:], in0=mk[:], in1=i8f[:], op=ALU.mult,
                    )
                    nc.scalar.activation(
                        out=junk[:], in_=junk[:], func=ACTF.Copy,
                        accum_out=lidxf[:, j:j + 1],
                    )
                # lidx = raw in-chunk pos + chunk(Pos)*C; chunk = round((Pos-3.5)/8)
                cbq = psc.tile([128, c.CAP], F32, tag="cbq", name="cbq")
                nc.vector.tensor_scalar(
                    out=cbq[:], in0=posf[:], scalar1=3.5, scalar2=0.125,
                    op0=ALU.subtract, op1=ALU.mult,
                )
                cbu = psc.tile([128, c.CAP], U32, tag="cbu", name="cbu")
                nc.vector.tensor_copy(out=cbu[:], in_=cbq[:])
                cbf = psc.tile([128, c.CAP], F32, tag="cbf", name="cbf")
                nc.vector.tensor_copy(out=cbf[:], in_=cbu[:])
                nc.vector.tensor_scalar(
                    out=cbf[:], in0=cbf[:], scalar1=float(c.C), scalar2=None,
                    op0=ALU.mult,
                )
                nc.vector.tensor_tensor(out=lidxf[:], in0=lidxf[:], in1=cbf[:], op=ALU.add)
                for j in range(c.CAP):
                    nc.vector.tensor_copy(out=lidx[mt][:, j:j + 1], in_=lidxf[:, j:j + 1])

            # ---- Stage C2: gather own candidate rows (fp32) + exact rescore ----
            rows = [pp.tile([128, c.CAP, c.D], F32, tag=f"rows{mt}", name=f"rows{mt}") for mt in range(c.MT)]
            vex = [pp.tile([128, c.CAP], F32, tag=f"vex{mt}", name=f"vex{mt}") for mt in range(c.MT)]
            for mt in range(c.MT):
                for j in range(c.CAP):
                    nc.gpsimd.indirect_dma_start(
                        out=rows[mt][:, j, :], out_offset=None, in_=ET[:],
                        in_offset=bass.IndirectOffsetOnAxis(
                            ap=lidx[mt][:, j:j + 1], axis=0
                        ),
                    )
                    prod = pg.tile([128, c.D], F32, tag="prod", name="prod")
                    nc.vector.tensor_tensor(
                        out=prod[:], in0=rows[mt][:, j, :], in1=pseudo_sb[mt][:],
                        op=ALU.mult,
                    )
                    nc.scalar.activation(
                        out=prod[:], in_=prod[:], func=ACTF.Copy,
                        accum_out=vex[mt][:, j:j + 1],
                    )

            # ---- Stage D: AllGather local top-K values ----
            lv_dram = pd.tile([c.M, c.AGK], F32)
            for mt in range(c.MT):
                nc.sync.dma_start(
                    out=lv_dram[mt * 128:(mt + 1) * 128, :], in_=vex[mt][:, :c.AGK]
                )
            gv_dram = pd.tile([c.NCORE * c.M, c.AGK], F32)
            nc.gpsimd.collective_compute(
                "AllGather", ALU.bypass, replica_groups=groups,
                ins=[lv_dram[:]], outs=[gv_dram[:]],
            )

            # ---- Stage E: per-mention stats + alpha for own top-CAP ----
            NK = c.NCORE * c.AGK
            ROUNDS_E = (c.K - 4) // 8   # knock 96, then ranks 97..104
            TAU_IDX = c.K - 8 * ROUNDS_E - 1
            alpha = [pp.tile([128, c.CAP], F32, tag=f"alpha{mt}", name=f"alpha{mt}") for mt in range(c.MT)]
            for mt in range(c.MT):
                gvs = psc.tile([128, c.NCORE, c.AGK], F32, tag="gvs", name="gvs")
                nc.sync.dma_start(
                    out=gvs[:],
                    in_=gv_dram[:].rearrange("(r m) s -> m r s", r=c.NCORE)[
                        mt * 128:(mt + 1) * 128
                    ],
                )
                gv2 = gvs[:].rearrange("p r s -> p (r s)")
                gmax = psc.tile([128, 1], F32, tag="gmax", name="gmax")
                nc.vector.tensor_reduce(out=gmax[:], in_=gv2, axis=AX.X, op=ALU.max)
                work2 = psc.tile([128, NK], F32, tag="work2", name="work2")
                m8 = psc.tile([128, 8], F32, tag="m8", name="m8")
                src = gv2
                for r in range(ROUNDS_E):
                    nc.vector.max(out=m8[:], in_=src)
                    nc.vector.match_replace(
                        out=work2[:], in_to_replace=m8[:], in_values=src, imm_value=NEG
                    )
                    src = work2[:]
                nc.vector.max(out=m8[:], in_=src)  # ranks 97..104
                tau = psc.tile([128, 1], F32, tag="tau", name="tau")
                nc.vector.tensor_copy(out=tau[:], in_=m8[:, TAU_IDX:TAU_IDX + 1])

                sub = psc.tile([128, NK], F32, tag="sub", name="sub")
                nc.vector.tensor_scalar(
                    out=sub[:], in0=gv2, scalar1=gmax[:, 0:1], scalar2=None,
                    op0=ALU.subtract,
                )
                e800 = psc.tile([128, NK], F32, tag="e800", name="e800")
                nc.scalar.activation(out=e800[:], in_=sub[:], func=ACTF.Exp)
                mask = psc.tile([128, NK], F32, tag="mask800", name="mask800")
                nc.vector.tensor_scalar(
                    out=mask[:], in0=gv2, scalar1=tau[:, 0:1], scalar2=None,
                    op0=ALU.is_ge,
                )
                nc.vector.tensor_tensor(out=e800[:], in0=e800[:], in1=mask[:], op=ALU.mult)
                denom = psc.tile([128, 1], F32, tag="denom", name="denom")
                nc.vector.tensor_reduce(out=denom[:], in_=e800[:], axis=AX.X, op=ALU.add)
                rden = psc.tile([128, 1], F32, tag="rden", name="rden")
                nc.vector.reciprocal(out=rden[:], in_=denom[:])

                own = vex[mt][:]
                a_s = psc.tile([128, c.CAP], F32, tag="a_s", name="a_s")
                nc.vector.tensor_scalar(
                    out=a_s[:], in0=own, scalar1=gmax[:, 0:1], scalar2=None,
                    op0=ALU.subtract,
                )
                a_e = psc.tile([128, c.CAP], F32, tag="a_e", name="a_e")
                nc.scalar.activation(out=a_e[:], in_=a_s[:], func=ACTF.Exp)
                a_m = psc.tile([128, c.CAP], F32, tag="a_m", name="a_m")
                nc.vector.tensor_scalar(
                    out=a_m[:], in0=own, scalar1=tau[:, 0:1], scalar2=None, op0=ALU.is_ge,
                )
                nc.vector.tensor_tensor(out=a_e[:], in0=a_e[:], in1=a_m[:], op=ALU.mult)
                nc.vector.tensor_scalar(
                    out=alpha[mt][:], in0=a_e[:], scalar1=rden[:, 0:1], scalar2=None,
                    op0=ALU.mult,
                )

            # ---- Stage F: gather own top-CAP rows (bf16), weighted accumulate ----
            acc = [pp.tile([128, c.D], F32, tag=f"acc{mt}", name=f"acc{mt}") for mt in range(c.MT)]
            for mt in range(c.MT):
                nc.vector.memset(acc[mt][:], 0.0)
                for j in range(c.CAP):
                    tmp = pg.tile([128, c.D], F32, tag="tmp", name="tmp")
                    nc.vector.tensor_scalar(
                        out=tmp[:], in0=rows[mt][:, j, :], scalar1=alpha[mt][:, j:j + 1],
                        scalar2=None, op0=ALU.mult,
                    )
                    nc.vector.tensor_tensor(
                        out=acc[mt][:], in0=acc[mt][:], in1=tmp[:], op=ALU.add
                    )

            if debug:
                for mt in range(c.MT):
                    sl = slice(mt * 128, (mt + 1) * 128)
                    nc.sync.dma_start(out=dbg_lv[sl, :], in_=W[mt][:])
                    nc.sync.dma_start(out=dbg_lidx[sl, :], in_=lidx[mt][:])
                    nc.sync.dma_start(out=dbg_alpha[sl, :], in_=alpha[mt][:])
                    nc.sync.dma_start(out=dbg_acc[sl, :], in_=acc[mt][:])
                    nc.sync.dma_start(out=dbg_pos[sl, :], in_=Pos[mt][:])
                    nc.sync.dma_start(out=dbg_i8[sl, :], in_=I8[mt][:])

            # ---- Stage G: AllReduce partial picked ----
            pk_dram = pd.tile([c.M, c.D], F32)
            for mt in range(c.MT):
                nc.sync.dma_start(
                    out=pk_dram[mt * 128:(mt + 1) * 128, :], in_=acc[mt][:]
                )
            ar_dram = pd.tile([c.M, c.D], F32)
            nc.gpsimd.collective_compute(
                "AllReduce", ALU.add, replica_groups=groups,
                ins=[pk_dram[:]], outs=[ar_dram[:]],
            )
            picked_sb = [pp.tile([128, c.D], F32, tag=f"picked{mt}", name=f"picked{mt}") for mt in range(c.MT)]
            for mt in range(c.MT):
                nc.sync.dma_start(
                    out=picked_sb[mt][:], in_=ar_dram[mt * 128:(mt + 1) * 128, :]
                )

            # ---- Stage H: upd = picked @ Wb^T + b; scatter into y ----
            if debug:
                for mt in range(c.MT):
                    nc.sync.dma_start(out=dbg_pick[mt * 128:(mt + 1) * 128, :], in_=picked_sb[mt][:])
            pT_sb = [pp.tile([128, c.M], F32, tag=f"pT{dt}", name=f"pT{dt}") for dt in range(c.DT)]
            for dt in range(c.DT):
                for mt in range(c.MT):
                    pst = pps1.tile([128, 128], F32, tag="tr_ps2", name="tr_ps2")
                    nc.tensor.transpose(
                        out=pst[:],
                        in_=picked_sb[mt][:, dt * 128:(dt + 1) * 128],
                        identity=ident[:],
                    )
                    nc.vector.tensor_copy(
                        out=pT_sb[dt][:, mt * 128:(mt + 1) * 128], in_=pst[:]
                    )
            WbT_sb = pp.tile([128, c.DT, c.DEMB], F32)
            nc.sync.dma_start(
                out=WbT_sb[:], in_=WbT.rearrange("(t p) n -> p t n", p=128)
            )
            Wbb_sb = pp.tile([1, c.DEMB], F32)
            nc.sync.dma_start(out=Wbb_sb[:], in_=Wbb[:])

            NBW = c.DEMB // c.NB  # 384
            upd_sb = [pp.tile([128, c.DEMB], F32, tag=f"upd{mt}", name=f"upd{mt}") for mt in range(c.MT)]
            for mt in range(c.MT):
                for nb in range(c.NB):
                    pu = pps1.tile([128, NBW], F32, tag="upd_ps", name="upd_ps")
                    for dt in range(c.DT):
                        nc.tensor.matmul(
                            pu[:],
                            lhsT=pT_sb[dt][:, mt * 128:(mt + 1) * 128],
                            rhs=WbT_sb[:, dt, nb * NBW:(nb + 1) * NBW],
                            start=(dt == 0),
                            stop=False,
                        )
                    nc.tensor.matmul(
                        pu[:], lhsT=ones1[0:1, :],
                        rhs=Wbb_sb[0:1, nb * NBW:(nb + 1) * NBW],
                        start=False, stop=True,
                    )
                    nc.vector.tensor_copy(
                        out=upd_sb[mt][:, nb * NBW:(nb + 1) * NBW], in_=pu[:]
                    )
            for mt in range(c.MT):
                ridx = pp.tile([128, 1], U32, tag=f"ridx{mt}", name=f"ridx{mt}")
                nc.sync.dma_start(
                    out=ridx[:], in_=ROWIDX[mt * 128:(mt + 1) * 128, :]
                )
                nc.gpsimd.indirect_dma_start(
                    out=y[:], out_offset=bass.IndirectOffsetOnAxis(ap=ridx[:, 0:1], axis=0),
                    in_=upd_sb[mt][:], in_offset=None,
                    bounds_check=c.B * c.S - 1, oob_is_err=False,
                )

    nc.finalize()
    return nc


def _prep_inputs(X, mention_b, mention_begin, mention_end, Wf_w, Wf_b, Wb_w, Wb_b,
                 E_w, cfg):
    c = cfg
    X = np.asarray(X, np.float32)
    mb = np.asarray(mention_b).astype(np.int64)
    mbeg = np.asarray(mention_begin).astype(np.int64)
    mend = np.asarray(mention_end).astype(np.int64)
    E_w = np.ascontiguousarray(np.asarray(E_w, np.float32))

    first = X[mb, mbeg]
    second = X[mb, mend]
    span = np.concatenate([first, second], axis=1).astype(np.float32)  # [M, 1536]
    common = {
        "IOTA": np.tile(np.arange(c.NL // c.C * 8, dtype=np.float32), (128, 1)),
        "spanT": np.ascontiguousarray(span.T),
        "WfT": np.ascontiguousarray(np.asarray(Wf_w, np.float32).T),
        "Wfb": np.asarray(Wf_b, np.float32).reshape(1, -1).copy(),
        "WbT": np.ascontiguousarray(np.asarray(Wb_w, np.float32).T),
        "Wbb": np.asarray(Wb_b, np.float32).reshape(1, -1).copy(),
    }
    in_maps = []
    for ci in range(c.NCORE):
        sl = E_w[:, ci * c.NL:(ci + 1) * c.NL]
        m = dict(common)
        m["E"] = np.ascontiguousarray(sl)
        m["ET"] = np.ascontiguousarray(sl.T)
        in_maps.append(m)

    rows = (mb * c.S + mbeg).astype(np.int64)
    keep = {}
    for m_i in range(c.M):
        keep[int(rows[m_i])] = m_i  # duplicates: last mention wins (matches jax .set)
    winner = {m_i for m_i in keep.values()}
    rowidx = np.full((c.M, 1), 2**31, np.uint32)  # losers: out-of-bounds -> skipped
    for m_i in range(c.M):
        if m_i in winner:
            rowidx[m_i, 0] = rows[m_i]
    for mm in in_maps:
        mm["ROWIDX"] = rowidx
    scatter_items = sorted((r, m_i) for r, m_i in keep.items())
    return in_maps, scatter_items


def kernel(X, mention_b, mention_begin, mention_end, Wf_w, Wf_b, Wb_w, Wb_b,
           E_w, k, **_unused):
    cfg = Cfg()
    assert int(np.asarray(k)) == cfg.K
    in_maps, scatter_items = _prep_inputs(
        X, mention_b, mention_begin, mention_end, Wf_w, Wf_b, Wb_w, Wb_b, E_w, cfg
    )
    nc = build(cfg, scatter_items)
    trace = bool(os.environ.get("EAE_TRACE"))
    res = run_bass_kernel_spmd(nc, in_maps, list(range(cfg.NCORE)), trace=trace)
    global _LAST_RESULT
    _LAST_RESULT = res
    y = np.asarray(res.results[0]["y"], np.float32).reshape(cfg.B, cfg.S, cfg.DEMB)
    return y


_LAST_RESULT = None


# revision 24
# speedup vs baseline: 1.1295x; 1.1295x over previous
"""EntitiesAsExperts retrieval kernel for 8 Trainium2 NeuronCores.

Algorithm (entity table sharded along N across 8 cores):
  - host: gather mention spans from X, transpose small weights
  - device (SPMD, per core):
      pseudo = span @ Wf^T + b          (replicated small matmul)
      stream E shard in [128, C] chunks: scores chunk = pseudoT^T @ E_chunk (PE, fp32)
        per chunk: top-8 values (DVE max8) + their in-chunk positions (max_index)
      local top-104 of the per-chunk top-8s (13 rounds max8/match_replace)
      AllGather local top-100 values -> [8*256, 100]
      per mention: global max, 100th-largest (tau), softmax denominator over
        winners (v >= tau); alpha for own top-32 local candidates
      gather own winning entity rows (bf16 copy of shard, indirect DMA),
        weighted-accumulate -> partial picked; AllReduce picked
      upd = picked @ Wb^T + b; scatter rows into zeroed y at baked offsets
  - host: take core 0's y
"""

import os
import numpy as np
import ml_dtypes

import concourse.bass as bass
import concourse.bacc as bacc
import concourse.mybir as mybir
import concourse.tile as tile
from concourse.bass_utils import run_bass_kernel_spmd
from concourse.masks import make_identity

F32 = mybir.dt.float32
BF16 = mybir.dt.bfloat16
U16 = mybir.dt.uint16
U32 = mybir.dt.uint32
AX = mybir.AxisListType
ALU = mybir.AluOpType
ACTF = mybir.ActivationFunctionType

NEG = -1.0e30


class Cfg:
    def __init__(self, NCORE=8, B=8, S=512, DEMB=768, D=256, M=256, NL=62500,
                 C=500, K=100, CAP=28):
        self.NCORE, self.B, self.S, self.DEMB, self.D, self.M = NCORE, B, S, DEMB, D, M
        self.NL, self.C, self.K, self.CAP = NL, C, K, CAP
        self.AGK = CAP  # AllGather ships the rescored top-CAP; winners/core <= 26
        assert NL % C == 0
        self.G = NL // C
        self.G8 = self.G * 8
        self.MT = M // 128
        self.DT = D // 128
        self.ROUNDS = (max(self.AGK, CAP) + 7) // 8  # 4 -> 32 slots
        assert self.ROUNDS * 8 <= self.G8
        self.SPAN = 2 * DEMB  # 1536
        self.KT1 = self.SPAN // 128  # 12
        assert DEMB % 2 == 0
        self.NB = 2  # Wb output split
        assert CAP % 2 == 0 and CAP <= K


def build(cfg: Cfg, scatter_items, debug=False):
    """scatter_items: list of (row, mention) with unique rows (deduped, last-wins)."""
    c = cfg
    nc = bacc.Bacc(num_devices=c.NCORE)

    spanT = nc.declare_dram_parameter("spanT", [c.SPAN, c.M], F32, isOutput=False)
    WfT = nc.declare_dram_parameter("WfT", [c.SPAN, c.D], F32, isOutput=False)
    Wfb = nc.declare_dram_parameter("Wfb", [1, c.D], F32, isOutput=False)
    WbT = nc.declare_dram_parameter("WbT", [c.D, c.DEMB], F32, isOutput=False)
    Wbb = nc.declare_dram_parameter("Wbb", [1, c.DEMB], F32, isOutput=False)
    E = nc.declare_dram_parameter("E", [c.D, c.NL], F32, isOutput=False)
    ET = nc.declare_dram_parameter("ET", [c.NL, c.D], F32, isOutput=False)
    IOTA = nc.declare_dram_parameter("IOTA", [128, c.G8], F32, isOutput=False)
    ROWIDX = nc.declare_dram_parameter("ROWIDX", [c.M, 1], U32, isOutput=False)
    y = nc.declare_dram_parameter("y", [c.B * c.S, c.DEMB], F32, isOutput=True)
    if debug:
        dbg_lv = nc.declare_dram_parameter("dbg_lv", [c.M, c.ROUNDS * 8], F32, isOutput=True)
        dbg_lidx = nc.declare_dram_parameter("dbg_lidx", [c.M, c.CAP], U32, isOutput=True)
        dbg_alpha = nc.declare_dram_parameter("dbg_alpha", [c.M, c.CAP], F32, isOutput=True)
        dbg_acc = nc.declare_dram_parameter("dbg_acc", [c.M, c.D], F32, isOutput=True)
        dbg_pick = nc.declare_dram_parameter("dbg_pick", [c.M, c.D], F32, isOutput=True)
        dbg_pos = nc.declare_dram_parameter("dbg_pos", [c.M, c.ROUNDS * 8], U16, isOutput=True)
        dbg_i8 = nc.declare_dram_parameter("dbg_i8", [c.M, 8 * (62500 // 500 if c.NL == 62500 else c.NL // c.C)], U32, isOutput=True)

    groups = [list(range(c.NCORE))]

    with tile.TileContext(nc) as tc:
        with (
            tc.tile_pool(name="persist", bufs=1) as pp,
            tc.tile_pool(name="estream", bufs=3) as pe,
            tc.tile_pool(name="scratch", bufs=2) as psc,
            tc.tile_pool(name="psum", bufs=2, space="PSUM") as pps,
            tc.tile_pool(name="psum1", bufs=1, space="PSUM") as pps1,
            tc.tile_pool(name="dram", bufs=1, space="DRAM") as pd,
            tc.tile_pool(name="gather", bufs=2) as pg,
        ):
            ident = pp.tile([128, 128], F32)
            make_identity(nc, ident[:])
            ones1 = pp.tile([1, 128], F32)
            nc.vector.memset(ones1[:], 1.0)

            # ---- Stage A: pseudo = span @ Wf^T + b; pseudoT ----
            spanT_sb = [pp.tile([128, c.M], F32, tag=f"spanT{t}", name=f"spanT{t}") for t in range(c.KT1)]
            WfT_sb = [pp.tile([128, c.D], F32, tag=f"WfTs{t}", name=f"WfTs{t}") for t in range(c.KT1)]
            for t in range(c.KT1):
                spr = psc.tile([128, c.M], F32, tag="spr", name="spr")
                nc.sync.dma_start(out=spr[:], in_=spanT[t * 128:(t + 1) * 128, :])
                nc.vector.tensor_copy(out=spanT_sb[t][:], in_=spr[:])
                nc.sync.dma_start(out=WfT_sb[t][:], in_=WfT[t * 128:(t + 1) * 128, :])
            Wfb_sb = pp.tile([1, c.D], F32)
            nc.sync.dma_start(out=Wfb_sb[:], in_=Wfb[:])

            pseudo_sb = [pp.tile([128, c.D], F32, tag=f"pseudo{mt}", name=f"pseudo{mt}") for mt in range(c.MT)]
            for mt in range(c.MT):
                ps = pps1.tile([128, c.D], F32, tag="pseudo_ps", name="pseudo_ps")
                for t in range(c.KT1):
                    nc.tensor.matmul(
                        ps[:],
                        lhsT=spanT_sb[t][:, mt * 128:(mt + 1) * 128],
                        rhs=WfT_sb[t][:],
                        start=(t == 0),
                        stop=False,
                    )
                nc.tensor.matmul(
                    ps[:], lhsT=ones1[0:1, :], rhs=Wfb_sb[0:1, :],
                    start=False, stop=True,
                )
                nc.vector.tensor_copy(out=pseudo_sb[mt][:], in_=ps[:])

            pseudoT_sb = [pp.tile([128, c.M], mybir.dt.float32r, tag=f"pseudoT{dt}", name=f"pseudoT{dt}") for dt in range(c.DT)]
            for dt in range(c.DT):
                for mt in range(c.MT):
                    pst = pps1.tile([128, 128], F32, tag="tr_ps", name="tr_ps")
                    nc.tensor.transpose(
                        out=pst[:],
                        in_=pseudo_sb[mt][:, dt * 128:(dt + 1) * 128],
                        identity=ident[:],
                    )
                    nc.vector.tensor_copy(
                        out=pseudoT_sb[dt][:, mt * 128:(mt + 1) * 128], in_=pst[:]
                    )

            # ---- Stage B: stream E chunks; per-chunk top-8 + positions ----
            V8 = [pp.tile([128, c.G8], F32, tag=f"V8_{mt}", name=f"V8_{mt}") for mt in range(c.MT)]
            I8 = [pp.tile([128, c.G8], U32, tag=f"I8_{mt}", name=f"I8_{mt}") for mt in range(c.MT)]
            for ci in range(c.G):
                e0 = pe.tile([128, c.C], F32, tag="e0", name="e0")
                nc.sync.dma_start(out=e0[:], in_=E[0:128, ci * c.C:(ci + 1) * c.C])
                e1 = pe.tile([128, c.C], F32, tag="e1", name="e1")
                nc.sync.dma_start(out=e1[:], in_=E[128:256, ci * c.C:(ci + 1) * c.C])
                e0r = pe.tile([128, c.C], mybir.dt.float32r, tag="e0r", name="e0r")
                nc.scalar.copy(out=e0r[:], in_=e0[:])
                e1r = pe.tile([128, c.C], mybir.dt.float32r, tag="e1r", name="e1r")
                nc.scalar.copy(out=e1r[:], in_=e1[:])
                for mt in range(c.MT):
                    ps = pps.tile([128, c.C], F32, tag=f"score{mt}", name=f"score{mt}")
                    nc.tensor.matmul(
                        ps[:], lhsT=pseudoT_sb[0][:, mt * 128:(mt + 1) * 128],
                        rhs=e0r[:], start=True, stop=False,
                    )
                    nc.tensor.matmul(
                        ps[:], lhsT=pseudoT_sb[1][:, mt * 128:(mt + 1) * 128],
                        rhs=e1r[:], start=False, stop=True,
                    )
                    v8s = V8[mt][:, ci * 8:(ci + 1) * 8]
                    nc.vector.max(out=v8s, in_=ps[:])
                    nc.vector.max_index(
                        out=I8[mt][:, ci * 8:(ci + 1) * 8], in_max=v8s, in_values=ps[:]
                    )

            # ---- Stage C: local top-104 of V8; positions; top-CAP indices ----
            iotaf = pp.tile([128, c.G8], F32)
            nc.sync.dma_start(out=iotaf[:], in_=IOTA[:])
            W = [pp.tile([128, c.ROUNDS * 8], F32, tag=f"W{mt}", name=f"W{mt}") for mt in range(c.MT)]
            Pos = [pp.tile([128, c.ROUNDS * 8], U16, tag=f"Pos{mt}", name=f"Pos{mt}") for mt in range(c.MT)]
            lidx = [pp.tile([128, c.CAP], U32, tag=f"lidx{mt}", name=f"lidx{mt}") for mt in range(c.MT)]
            for mt in range(c.MT):
                work = pp.tile([128, c.G8], F32, tag=f"work{mt}", name=f"work{mt}")
                for r in range(c.ROUNDS):
                    src = V8[mt][:] if r == 0 else work[:]
                    w8 = W[mt][:, r * 8:(r + 1) * 8]
                    nc.vector.max(out=w8, in_=src)
                    nc.vector.max_index(out=Pos[mt][:, r * 8:(r + 1) * 8], in_max=w8, in_values=src)
                    if r < c.ROUNDS - 1:
                        nc.vector.match_replace(
                            out=work[:], in_to_replace=w8, in_values=src, imm_value=NEG
                        )
                # gather I8 at the top-CAP slots: per j, one-hot match on slot
                # index then reduce (indirect_copy shares idxs per 16-partition
                # group, so it cannot do per-mention gathers)
                i8f = pp.tile([128, c.G8], F32, tag=f"i8f{mt}", name=f"i8f{mt}")
                nc.vector.tensor_copy(out=i8f[:], in_=I8[mt][:])
                posf = pp.tile([128, c.CAP], F32, tag=f"posf{mt}", name=f"posf{mt}")
                nc.vector.tensor_copy(out=posf[:], in_=Pos[mt][:, :c.CAP])
                # chunk base first: chunk = round((Pos-3.5)/8), base = chunk*C
                cbq = psc.tile([128, c.CAP], F32, tag="cbq", name="cbq")
                nc.vector.tensor_scalar(
                    out=cbq[:], in0=posf[:], scalar1=3.5, scalar2=0.125,
                    op0=ALU.subtract, op1=ALU.mult,
                )
                cbu = psc.tile([128, c.CAP], U32, tag="cbu", name="cbu")
                nc.vector.tensor_copy(out=cbu[:], in_=cbq[:])
                cbf = psc.tile([128, c.CAP], F32, tag="cbf", name="cbf")
                nc.vector.tensor_copy(out=cbf[:], in_=cbu[:])
                nc.vector.tensor_scalar(
                    out=cbf[:], in0=cbf[:], scalar1=float(c.C), scalar2=None,
                    op0=ALU.mult,
                )
                lidxf = pp.tile([128, c.CAP], F32, tag=f"lidxf{mt}", name=f"lidxf{mt}")
                for j in range(c.CAP):
                    mk = psc.tile([128, c.G8], F32, tag="mk", name="mk")
                    nc.vector.tensor_tensor(
                        out=mk[:], in0=iotaf[:],
                        in1=posf[:, j:j + 1].to_broadcast([128, c.G8]),
                        op=ALU.is_equal,
                    )
                    junk = psc.tile([128, c.G8], F32, tag="junk", name="junk")
                    nc.vector.tensor_tensor(
                        out=junk[:], in0=mk[:], in1=i8f[:], op=ALU.mult,
                    )
                    nc.scalar.activation(
                        out=junk[:], in_=junk[:], func=ACTF.Copy,
                        accum_out=lidxf[:, j:j + 1],
                    )
                    nc.vector.tensor_tensor(
                        out=lidxf[:, j:j + 1], in0=lidxf[:, j:j + 1],
                        in1=cbf[:, j:j + 1], op=ALU.add,
                    )
                    nc.vector.tensor_copy(out=lidx[mt][:, j:j + 1], in_=lidxf[:, j:j + 1])

            # ---- Stage C2: gather own candidate rows (fp32) + exact rescore ----
            rows = [pp.tile([128, c.CAP, c.D], F32, tag=f"rows{mt}", name=f"rows{mt}") for mt in range(c.MT)]
            vex = [pp.tile([128, c.CAP], F32, tag=f"vex{mt}", name=f"vex{mt}") for mt in range(c.MT)]
            for mt in range(c.MT):
                for j in range(c.CAP):
                    nc.gpsimd.indirect_dma_start(
                        out=rows[mt][:, j, :], out_offset=None, in_=ET[:],
                        in_offset=bass.IndirectOffsetOnAxis(
                            ap=lidx[mt][:, j:j + 1], axis=0
                        ),
                    )
                    prod = pg.tile([128, c.D], F32, tag="prod", name="prod")
                    nc.vector.tensor_tensor(
                        out=prod[:], in0=rows[mt][:, j, :], in1=pseudo_sb[mt][:],
                        op=ALU.mult,
                    )
                    nc.scalar.activation(
                        out=prod[:], in_=prod[:], func=ACTF.Copy,
                        accum_out=vex[mt][:, j:j + 1],
                    )

            # ---- Stage D: AllGather local top-K values ----
            lv_dram = pd.tile([c.M, c.AGK], F32)
            for mt in range(c.MT):
                nc.sync.dma_start(
                    out=lv_dram[mt * 128:(mt + 1) * 128, :], in_=vex[mt][:, :c.AGK]
                )
            gv_dram = pd.tile([c.NCORE * c.M, c.AGK], F32)
            nc.gpsimd.collective_compute(
                "AllGather", ALU.bypass, replica_groups=groups,
                ins=[lv_dram[:]], outs=[gv_dram[:]],
            )

            # ---- Stage E: per-mention stats + alpha for own top-CAP ----
            NK = c.NCORE * c.AGK
            ROUNDS_E = (c.K - 4) // 8   # knock 96, then ranks 97..104
            TAU_IDX = c.K - 8 * ROUNDS_E - 1
            alpha = [pp.tile([128, c.CAP], F32, tag=f"alpha{mt}", name=f"alpha{mt}") for mt in range(c.MT)]
            for mt in range(c.MT):
                gvs = psc.tile([128, c.NCORE, c.AGK], F32, tag="gvs", name="gvs")
                nc.sync.dma_start(
                    out=gvs[:],
                    in_=gv_dram[:].rearrange("(r m) s -> m r s", r=c.NCORE)[
                        mt * 128:(mt + 1) * 128
                    ],
                )
                gv2 = gvs[:].rearrange("p r s -> p (r s)")
                gmax = psc.tile([128, 1], F32, tag="gmax", name="gmax")
                nc.vector.tensor_reduce(out=gmax[:], in_=gv2, axis=AX.X, op=ALU.max)
                work2 = psc.tile([128, NK], F32, tag="work2", name="work2")
                m8 = psc.tile([128, 8], F32, tag="m8", name="m8")
                src = gv2
                for r in range(ROUNDS_E):
                    nc.vector.max(out=m8[:], in_=src)
                    nc.vector.match_replace(
                        out=work2[:], in_to_replace=m8[:], in_values=src, imm_value=NEG
                    )
                    src = work2[:]
                nc.vector.max(out=m8[:], in_=src)  # ranks 97..104
                tau = psc.tile([128, 1], F32, tag="tau", name="tau")
                nc.vector.tensor_copy(out=tau[:], in_=m8[:, TAU_IDX:TAU_IDX + 1])

                sub = psc.tile([128, NK], F32, tag="sub", name="sub")
                nc.vector.tensor_scalar(
                    out=sub[:], in0=gv2, scalar1=gmax[:, 0:1], scalar2=None,
                    op0=ALU.subtract,
                )
                e800 = psc.tile([128, NK], F32, tag="e800", name="e800")
                nc.scalar.activation(out=e800[:], in_=sub[:], func=ACTF.Exp)
                mask = psc.tile([128, NK], F32, tag="mask800", name="mask800")
                nc.vector.tensor_scalar(
                    out=mask[:], in0=gv2, scalar1=tau[:, 0:1], scalar2=None,
                    op0=ALU.is_ge,
                )
                nc.vector.tensor_tensor(out=e800[:], in0=e800[:], in1=mask[:], op=ALU.mult)
                denom = psc.tile([128, 1], F32, tag="denom", name="denom")
                nc.vector.tensor_reduce(out=denom[:], in_=e800[:], axis=AX.X, op=ALU.add)
                rden = psc.tile([128, 1], F32, tag="rden", name="rden")
                nc.vector.reciprocal(out=rden[:], in_=denom[:])

                own = vex[mt][:]
                a_s = psc.tile([128, c.CAP], F32, tag="a_s", name="a_s")
                nc.vector.tensor_scalar(
                    out=a_s[:], in0=own, scalar1=gmax[:, 0:1], scalar2=None,
                    op0=ALU.subtract,
                )
                a_e = psc.tile([128, c.CAP], F32, tag="a_e", name="a_e")
                nc.scalar.activation(out=a_e[:], in_=a_s[:], func=ACTF.Exp)
                a_m = psc.tile([128, c.CAP], F32, tag="a_m", name="a_m")
                nc.vector.tensor_scalar(
                    out=a_m[:], in0=own, scalar1=tau[:, 0:1], scalar2=None, op0=ALU.is_ge,
                )
                nc.vector.tensor_tensor(out=a_e[:], in0=a_e[:], in1=a_m[:], op=ALU.mult)
                nc.vector.tensor_scalar(
                    out=alpha[mt][:], in0=a_e[:], scalar1=rden[:, 0:1], scalar2=None,
                    op0=ALU.mult,
                )

            # ---- Stage F: gather own top-CAP rows (bf16), weighted accumulate ----
            acc = [pp.tile([128, c.D], F32, tag=f"acc{mt}", name=f"acc{mt}") for mt in range(c.MT)]
            for mt in range(c.MT):
                nc.vector.memset(acc[mt][:], 0.0)
                for j in range(c.CAP):
                    tmp = pg.tile([128, c.D], F32, tag="tmp", name="tmp")
                    nc.vector.tensor_scalar(
                        out=tmp[:], in0=rows[mt][:, j, :], scalar1=alpha[mt][:, j:j + 1],
                        scalar2=None, op0=ALU.mult,
                    )
                    nc.vector.tensor_tensor(
                        out=acc[mt][:], in0=acc[mt][:], in1=tmp[:], op=ALU.add
                    )

            if debug:
                for mt in range(c.MT):
                    sl = slice(mt * 128, (mt + 1) * 128)
                    nc.sync.dma_start(out=dbg_lv[sl, :], in_=W[mt][:])
                    nc.sync.dma_start(out=dbg_lidx[sl, :], in_=lidx[mt][:])
                    nc.sync.dma_start(out=dbg_alpha[sl, :], in_=alpha[mt][:])
                    nc.sync.dma_start(out=dbg_acc[sl, :], in_=acc[mt][:])
                    nc.sync.dma_start(out=dbg_pos[sl, :], in_=Pos[mt][:])
                    nc.sync.dma_start(out=dbg_i8[sl, :], in_=I8[mt][:])

            # ---- Stage G: AllReduce partial picked ----
            pk_dram = pd.tile([c.M, c.D], F32)
            for mt in range(c.MT):
                nc.sync.dma_start(
                    out=pk_dram[mt * 128:(mt + 1) * 128, :], in_=acc[mt][:]
                )
            ar_dram = pd.tile([c.M, c.D], F32)
            nc.gpsimd.collective_compute(
                "AllReduce", ALU.add, replica_groups=groups,
                ins=[pk_dram[:]], outs=[ar_dram[:]],
            )
            picked_sb = [pp.tile([128, c.D], F32, tag=f"picked{mt}", name=f"picked{mt}") for mt in range(c.MT)]
            for mt in range(c.MT):
                nc.sync.dma_start(
                    out=picked_sb[mt][:], in_=ar_dram[mt * 128:(mt + 1) * 128, :]
                )

            # ---- Stage H: upd = picked @ Wb^T + b; scatter into y ----
            if debug:
                for mt in range(c.MT):
                    nc.sync.dma_start(out=dbg_pick[mt * 128:(mt + 1) * 128, :], in_=picked_sb[mt][:])
            pT_sb = [pp.tile([128, c.M], F32, tag=f"pT{dt}", name=f"pT{dt}") for dt in range(c.DT)]
            for dt in range(c.DT):
                for mt in range(c.MT):
                    pst = pps1.tile([128, 128], F32, tag="tr_ps2", name="tr_ps2")
                    nc.tensor.transpose(
                        out=pst[:],
                        in_=picked_sb[mt][:, dt * 128:(dt + 1) * 128],
                        identity=ident[:],
                    )
                    nc.vector.tensor_copy(
                        out=pT_sb[dt][:, mt * 128:(mt + 1) * 128], in_=pst[:]
                    )
            WbT_sb = pp.tile([128, c.DT, c.DEMB], F32)
            nc.sync.dma_start(
                out=WbT_sb[:], in_=WbT.rearrange("(t p) n -> p t n", p=128)
            )
            Wbb_sb = pp.tile([1, c.DEMB], F32)
            nc.sync.dma_start(out=Wbb_sb[:], in_=Wbb[:])

            NBW = c.DEMB // c.NB  # 384
            upd_sb = [pp.tile([128, c.DEMB], F32, tag=f"upd{mt}", name=f"upd{mt}") for mt in range(c.MT)]
            for mt in range(c.MT):
                for nb in range(c.NB):
                    pu = pps1.tile([128, NBW], F32, tag="upd_ps", name="upd_ps")
                    for dt in range(c.DT):
                        nc.tensor.matmul(
                            pu[:],
                            lhsT=pT_sb[dt][:, mt * 128:(mt + 1) * 128],
                            rhs=WbT_sb[:, dt, nb * NBW:(nb + 1) * NBW],
                            start=(dt == 0),
                            stop=False,
                        )
                    nc.tensor.matmul(
                        pu[:], lhsT=ones1[0:1, :],
                        rhs=Wbb_sb[0:1, nb * NBW:(nb + 1) * NBW],
                        start=False, stop=True,
                    )
                    nc.vector.tensor_copy(
                        out=upd_sb[mt][:, nb * NBW:(nb + 1) * NBW], in_=pu[:]
                    )
            for mt in range(c.MT):
                ridx = pp.tile([128, 1], U32, tag=f"ridx{mt}", name=f"ridx{mt}")
                nc.sync.dma_start(
                    out=ridx[:], in_=ROWIDX[mt * 128:(mt + 1) * 128, :]
                )
                nc.gpsimd.indirect_dma_start(
                    out=y[:], out_offset=bass.IndirectOffsetOnAxis(ap=ridx[:, 0:1], axis=0),
                    in_=upd_sb[mt][:], in_offset=None,
                    bounds_check=c.B * c.S - 1, oob_is_err=False,
                )

    nc.finalize()
    return nc


def _prep_inputs(X, mention_b, mention_begin, mention_end, Wf_w, Wf_b, Wb_w, Wb_b,
                 E_w, cfg):
    c = cfg
    X = np.asarray(X, np.float32)
    mb = np.asarray(mention_b).astype(np.int64)
    mbeg = np.asarray(mention_begin).astype(np.int64)
    mend = np.asarray(mention_end).astype(np.int64)
    E_w = np.ascontiguousarray(np.asarray(E_w, np.float32))

    first = X[mb, mbeg]
    second = X[mb, mend]
    span = np.concatenate([first, second], axis=1).astype(np.float32)  # [M, 1536]
    common = {
        "IOTA": np.tile(np.arange(c.NL // c.C * 8, dtype=np.float32), (128, 1)),
        "spanT": np.ascontiguousarray(span.T),
        "WfT": np.ascontiguousarray(np.asarray(Wf_w, np.float32).T),
        "Wfb": np.asarray(Wf_b, np.float32).reshape(1, -1).copy(),
        "WbT": np.ascontiguousarray(np.asarray(Wb_w, np.float32).T),
        "Wbb": np.asarray(Wb_b, np.float32).reshape(1, -1).copy(),
    }
    in_maps = []
    for ci in range(c.NCORE):
        sl = E_w[:, ci * c.NL:(ci + 1) * c.NL]
        m = dict(common)
        m["E"] = np.ascontiguousarray(sl)
        m["ET"] = np.ascontiguousarray(sl.T)
        in_maps.append(m)

    rows = (mb * c.S + mbeg).astype(np.int64)
    keep = {}
    for m_i in range(c.M):
        keep[int(rows[m_i])] = m_i  # duplicates: last mention wins (matches jax .set)
    winner = {m_i for m_i in keep.values()}
    rowidx = np.full((c.M, 1), 2**31, np.uint32)  # losers: out-of-bounds -> skipped
    for m_i in range(c.M):
        if m_i in winner:
            rowidx[m_i, 0] = rows[m_i]
    for mm in in_maps:
        mm["ROWIDX"] = rowidx
    scatter_items = sorted((r, m_i) for r, m_i in keep.items())
    return in_maps, scatter_items


def kernel(X, mention_b, mention_begin, mention_end, Wf_w, Wf_b, Wb_w, Wb_b,
           E_w, k, **_unused):
    cfg = Cfg()
    assert int(np.asarray(k)) == cfg.K
    in_maps, scatter_items = _prep_inputs(
        X, mention_b, mention_begin, mention_end, Wf_w, Wf_b, Wb_w, Wb_b, E_w, cfg
    )
    nc = build(cfg, scatter_items)
    trace = bool(os.environ.get("EAE_TRACE"))
    res = run_bass_kernel_spmd(nc, in_maps, list(range(cfg.NCORE)), trace=trace)
    global _LAST_RESULT
    _LAST_RESULT = res
    y = np.asarray(res.results[0]["y"], np.float32).reshape(cfg.B, cfg.S, cfg.DEMB)
    return y


_LAST_RESULT = None
